# revision 12
# baseline (speedup 1.0000x reference)
"""Causal self-attention (B=2, T=2048, C=1024, H=16) on 8 trn2 NeuronCores.

Sharding: batch x head-group. Core c handles batch b = c//4 and heads
[4*(c%4), 4*(c%4)+4). Each core computes qkv for its head slice, causal
attention, and a partial c_proj ([T, C] over its 256 input rows of W_proj);
the host sums the 4 fp16 partials per batch in fp32.

v4 schedule. Two co-critical resources: the ACT engine's exp stream (~80us:
1 elem/lane/cycle @1.2GHz over the 8.9M causal logits, irreducible) and the
PE's ~117us of bf16 matmul columns.  Design:
  - attention blocks run as two-head PAIRS with j-steps interleaved
    (S_A S_B | O_A O_B | braid): each block's S/O covers the partner's exp
    latency, keeping the PE stream dense so the HAM clock stays at 2.4GHz
    through the exp-paced back half;
  - softmax normalize without ACT and without slow reciprocals: the O
    accumulation carries [denom; zeros] in psum rows 64:96 (vaug columns
    64:96 = [1; 0...]), DVE-transposes them to put the 512 denominators
    across 32 partitions (reciprocal is ~6.5 cyc/elem on the free dim, so
    a [1,512] row costs 3.3us but the [32,16] spread costs 0.15us), then a
    dram round-trip broadcast feeds the [64,512] multiply; bank0 of each O
    group is final at j=8m+3, so its half normalizes four j-steps early;
  - c_proj is braided into the pairs as 512-col psum half-tiles: tiles 0..7
    both-chunk halves into pair (2,3|m=1), tiles 8..15 as chunk-1 stages
    into pair (0,1|m=1) plus chunk-0 finishes at the tail; output is fp16;
  - input x rides the sync/scalar queues in 512-col quarters (first mass =
    1MiB x + 2MiB weights at the ~270GB/s effective DMA roofline), weights
    rotate over sync/scalar/gpsimd; a ~7us junk burst covers the lead-in;
  - all psum->sbuf movement rides DVE (gpsimd has no PSUM port); the
    gpsimd queue carries the braided output DMAs, keeping ACT exp-only.
"""

import contextlib
import functools
import sys

sys.path.insert(0, "/opt/trn_rl_repo")

import numpy as np

import concourse.bacc as bacc
import concourse.mybir as mybir
import concourse.tile as tile
from concourse import bass_utils
from concourse.alu_op_type import AluOpType

B, T, C, H, D = 2, 2048, 1024, 16, 64
NEG = -1e10
NCORES = 8
HEADS_PER_CORE = 4
DLOC = HEADS_PER_CORE * D  # 256 local head dims per core
F32 = mybir.dt.float32
F16 = mybir.dt.float16
BF16 = mybir.dt.bfloat16
AF = mybir.ActivationFunctionType

IN_DT = BF16
OUT_DT = F16
# ~duration of junk pre-warm matmuls covering the input-DMA lead-in (ns)
WARM_NS = 7000

NTB = T // 512  # 4 t-blocks in qkv phase
NKC = T // 128  # 16 k-chunks
VAUG_W = 96  # v cols 0:64, ones col 64, zero cols 65:96 (psum transposable)


def _pieces(a, end=1024):
    """Split [a, end) at 512-boundaries (psum bank boundaries)."""
    cuts = [a]
    b = (a // 512 + 1) * 512
    while b < end:
        cuts.append(b)
        b += 512
    cuts.append(end)
    return list(zip(cuts[:-1], cuts[1:]))


class Ctx:
    pass


def _emit_prewarm(nc, g):
    """Dependency-free fp32 matmuls covering the input-DMA lead-in so the
    PE hands off at full HAM clock to the dense qkv stream."""
    n = 3 + max(0, int((WARM_NS - 5200) / 880))
    ps = g.pool_x.tile([128, 1024], F32, tag="px", name="warm_ps")
    for i in range(n):
        nc.tensor.matmul(
            ps[:, 0:512],
            g.warm_sb[:, 0:128],
            g.warm_sb,
            start=(i == 0),
            stop=(i == n - 1),
        )
    wsink = g.rnpool.tile([1, 128], F32, tag="wsink", name="wsink", bufs=1)
    nc.vector.tensor_copy(wsink, ps[0:1, 0:128])
    nc.sync.dma_start(out=g.rn_dram.ap()[31:32, 0:128], in_=wsink)


def _emit_qkv_tblock(nc, g, tb):
    """qkv projections for t in [tb*512, (tb+1)*512), chunk-outer.

    v chains for ts 2,3 (psvB) ride the pool_o slot of the PREVIOUS
    attention block's pso, which frees only after its normalize chain:
    they run in a second pass so the PE reaches them ~5us in.
    """
    psq = g.pool_x.tile([128, 1024], F32, tag="px", name="psq")
    psk = g.pool_x.tile([128, 1024], F32, tag="px", name="psk")
    psv = [
        g.pool_o.tile([128, 1024], F32, tag="po", name="psvA"),
        g.pool_o.tile([128, 1024], F32, tag="po", name="psvB"),
    ]

    def vslice(ts, width=256):
        return psv[ts // 2][:, (ts % 2) * 512 : (ts % 2) * 512 + width]

    for cc in range(8):
        xts = g.x_sb[cc][:, tb * 512 : (tb + 1) * 512]
        st, sp = cc == 0, cc == 7
        for dt_ in range(2):
            nc.tensor.matmul(
                psq[:, dt_ * 512 : (dt_ + 1) * 512],
                g.wq_sb[cc][:, dt_ * 128 : (dt_ + 1) * 128],
                xts,
                start=st,
                stop=sp,
            )
            nc.tensor.matmul(
                psk[:, dt_ * 512 : (dt_ + 1) * 512],
                g.wk_sb[cc][:, dt_ * 128 : (dt_ + 1) * 128],
                xts,
                start=st,
                stop=sp,
            )
        for ts in range(2):
            nc.tensor.matmul(
                vslice(ts),
                xts[:, ts * 128 : (ts + 1) * 128],
                g.wv_sb[cc],
                start=st,
                stop=sp,
            )
    for cc in range(8):
        xts = g.x_sb[cc][:, tb * 512 : (tb + 1) * 512]
        st, sp = cc == 0, cc == 7
        for ts in range(2, 4):
            nc.tensor.matmul(
                vslice(ts),
                xts[:, ts * 128 : (ts + 1) * 128],
                g.wv_sb[cc],
                start=st,
                stop=sp,
            )
    for dt_ in range(2):
        nc.vector.tensor_scalar(
            out=g.qT_sb[:, dt_, tb * 512 : (tb + 1) * 512],
            in0=psq[:, dt_ * 512 : (dt_ + 1) * 512],
            scalar1=g.bq_sb[:, dt_ : dt_ + 1],
            scalar2=None,
            op0=AluOpType.add,
        )
        nc.vector.tensor_scalar(
            out=g.kT_sb[:, dt_, tb * 512 : (tb + 1) * 512],
            in0=psk[:, dt_ * 512 : (dt_ + 1) * 512],
            scalar1=g.bk_sb[:, dt_ : dt_ + 1],
            scalar2=None,
            op0=AluOpType.add,
        )
    for ts in range(4):
        kc = tb * 4 + ts
        for h in range(4):
            nc.vector.tensor_tensor(
                out=g.vaug[h][:, kc, 0:D],
                in0=vslice(ts)[:, h * D : (h + 1) * D],
                in1=g.bvb_sb[:, h * D : (h + 1) * D],
                op=AluOpType.add,
            )


class Block:
    """Emission state for one head x one 1024-wide q-block of attention."""

    def __init__(self, nc, g, h, m):
        self.nc, self.g, self.h, self.m = nc, g, h, m
        self.prow = (h % 2) * 64
        self.pi = h // 2
        self.njs = 8 * m + 8
        self.last_b0 = 8 * m + 3
        self.pso = g.pool_o.tile([128, 1024], F32, tag="po", name="pso")
        self.uts = {}

    def S_exp(self, j):
        nc, g = self.nc, self.g
        a = max(0, 128 * j - 1024 * self.m)
        pss = g.pool_x.tile([128, 1024], F32, tag="px", name="pss")
        for c0, c1 in _pieces(a):
            nc.tensor.matmul(
                pss[:, c0:c1],
                g.kT_sb[self.prow : self.prow + 64, self.pi, j * 128 : (j + 1) * 128],
                g.qT_sb[
                    self.prow : self.prow + 64,
                    self.pi,
                    self.m * 1024 + c0 : self.m * 1024 + c1,
                ],
                start=True,
                stop=True,
            )
        ut = g.utpool.tile([128, 1024], BF16, tag="ut", name="ut")
        self.uts[j] = ut
        nc.scalar.activation(
            out=ut[:, a:1024],
            in_=pss[:, a:1024],
            func=AF.Exp,
            bias=g.mneg_sb[:, j : j + 1],
            scale=0.125,
        )
        if j >= 8 * self.m:
            nc.vector.tensor_mul(ut[:, a : a + 128], ut[:, a : a + 128], g.tri_sb)

    def O(self, j):
        nc, g = self.nc, self.g
        a = max(0, 128 * j - 1024 * self.m)
        ut = self.uts.pop(j)
        for c0, c1 in _pieces(a):
            stop = j == (self.last_b0 if c0 < 512 else self.njs - 1)
            nc.tensor.matmul(
                self.pso[0:VAUG_W, c0:c1],
                g.vaug[self.h][:, j, :],
                ut[:, c0:c1],
                start=(j == 0),
                stop=stop,
            )

    def chain(self, half):
        """Start the 1/denom pipeline for psum bank `half` (cols 512*half+).

        psum rows 64:96 hold [denom; zeros]; transpose spreads the 512
        denominators over 32 partitions (16 per lane) so reciprocal is
        cheap; a dram round-trip broadcasts them to [64, 512]."""
        nc, g = self.nc, self.g
        c0 = half * 512
        row = (self.h * 2 + self.m) * 4 + half * 2  # 2 dram rows per chain
        dT = g.rnpool.tile([32, 512], F32, tag="dT", name="dT")
        nc.vector.transpose(dT, self.pso[64:96, c0 : c0 + 512])
        trow = g.rnpool.tile([32, 512], F32, tag="trow", name="trow")
        dT_s = dT.rearrange("p (b q) -> p b q", q=32)[:, :, 0]
        tr_s = trow.rearrange("p (b q) -> p b q", q=32)[:, :, 0]
        nc.vector.reciprocal(tr_s, dT_s)
        dst = g.rn_dram.ap()[row, 0:512].rearrange("(b p) -> p b", p=32)
        nc.sync.dma_start(out=dst, in_=tr_s)
        rnb = g.rnpool.tile([64, 512], F32, tag="rnb", name="rnb")
        nc.sync.dma_start(
            out=rnb,
            in_=g.rn_dram.ap()[row : row + 1, 0:512].partition_broadcast(64),
        )
        self._rnb = getattr(self, "_rnb", {})
        self._rnb[half] = rnb

    def mult(self, half):
        nc, g = self.nc, self.g
        c0 = half * 512
        nc.vector.tensor_tensor(
            out=g.yT_sb[
                self.prow : self.prow + 64,
                self.pi,
                self.m * 1024 + c0 : self.m * 1024 + c0 + 512,
            ],
            in0=self.pso[0:D, c0 : c0 + 512],
            in1=self._rnb.pop(half),
            op=AluOpType.mult,
        )


def _emit_attention_single(nc, g, h, m, braid=None):
    """One block alone (braided externally with qkv at the sequence level).
    bank0 chain starts at j=last_b0+1, its mult 3 steps later; bank1 at
    the end (the DVE wait hides behind the following qkv's bias lag)."""
    blk = Block(nc, g, h, m)
    braid = braid or {}
    blk.S_exp(0)
    for j in range(1, blk.njs):
        blk.S_exp(j)
        blk.O(j - 1)
        if j - 1 == blk.last_b0:
            blk.chain(0)
        if j == blk.last_b0 + 4:
            blk.mult(0)
        for fn in braid.get(j, ()):
            fn()
    blk.O(blk.njs - 1)
    blk.chain(1)
    blk.mult(1)
    for fn in braid.get(blk.njs, ()):
        fn()


def _emit_attention_pair(nc, g, hA, hB, m, braid=None):
    """Two heads' blocks with interleaved j-steps: each block's S/O covers
    the partner's exp latency so the PE stream stays dense."""
    A = Block(nc, g, hA, m)
    Bb = Block(nc, g, hB, m)
    braid = braid or {}
    A.S_exp(0)
    Bb.S_exp(0)
    for j in range(1, A.njs):
        A.S_exp(j)
        Bb.S_exp(j)
        A.O(j - 1)
        Bb.O(j - 1)
        if j - 1 == A.last_b0:
            A.chain(0)
            Bb.chain(0)
        if j == A.last_b0 + 4:
            A.mult(0)
            Bb.mult(0)
        for fn in braid.get(j, ()):
            fn()
    A.O(A.njs - 1)
    Bb.O(Bb.njs - 1)
    A.chain(1)
    Bb.chain(1)
    A.mult(1)
    Bb.mult(1)
    for fn in braid.get(A.njs, ()):
        fn()


def _spread(items, j0, j1):
    """Distribute items over j positions [j0, j1] -> dict j -> [item]."""
    out = {}
    n = len(items)
    span = j1 - j0 + 1
    for idx, it in enumerate(items):
        j = j0 + (idx * span) // n
        out.setdefault(j, []).append(it)
    return out


def _build(ctx, nc, tc, ins, out, rn_dram):
    g = Ctx()
    g.rn_dram = rn_dram

    singles = ctx.enter_context(tc.tile_pool(name="singles", bufs=1))
    g.pool_x = ctx.enter_context(tc.tile_pool(name="pool_x", bufs=2, space="PSUM"))
    g.pool_o = ctx.enter_context(tc.tile_pool(name="pool_o", bufs=2, space="PSUM"))
    g.utpool = ctx.enter_context(tc.tile_pool(name="utpool", bufs=6))
    g.rnpool = ctx.enter_context(tc.tile_pool(name="rnpool", bufs=2))
    g.outp = ctx.enter_context(tc.tile_pool(name="outp", bufs=3))

    # tri mask first (sync queue): the pre-warm burst depends only on memset
    g.tri_sb = singles.tile([128, 128], BF16, name="tri_sb")
    nc.sync.dma_start(out=g.tri_sb, in_=ins["tri"].ap())
    g.warm_sb = singles.tile([128, 512], F32, name="warm_sb")
    nc.vector.memset(g.warm_sb, 0.5)
    _emit_prewarm(nc, g)

    # --- inputs -----------------------------------------------------------
    # x quarters for tblocks 0,1 land first (the ~3MiB first-mass rides the
    # ~270GB/s effective DMA roofline); the t>=1024 halves follow. Weights
    # rotate over all three DMA-capable queues.
    q_sx = [nc.sync, nc.scalar]
    q3 = [nc.sync, nc.scalar, nc.gpsimd]

    g.x_sb = [singles.tile([128, T], IN_DT, name=f"x{c}") for c in range(8)]
    g.wq_sb = [singles.tile([128, DLOC], IN_DT, name=f"wq{c}") for c in range(8)]
    g.wk_sb = [singles.tile([128, DLOC], IN_DT, name=f"wk{c}") for c in range(8)]
    g.wv_sb = [singles.tile([128, DLOC], IN_DT, name=f"wv{c}") for c in range(8)]
    xT_r = ins["xT"].ap().rearrange("(c p) t -> p c t", p=128)
    wq_src = ins["wq"].ap().rearrange("(c p) m -> p c m", p=128)
    wk_src = ins["wk"].ap().rearrange("(c p) m -> p c m", p=128)
    wv_src = ins["wv"].ap().rearrange("(c p) m -> p c m", p=128)
    for cc in range(8):
        q_sx[cc % 2].dma_start(out=g.x_sb[cc][:, 0:512], in_=xT_r[:, cc, 0:512])
        q3[(cc + 0) % 3].dma_start(out=g.wq_sb[cc], in_=wq_src[:, cc, :])
        q3[(cc + 1) % 3].dma_start(out=g.wk_sb[cc], in_=wk_src[:, cc, :])
        q3[(cc + 2) % 3].dma_start(out=g.wv_sb[cc], in_=wv_src[:, cc, :])

    g.bq_sb = singles.tile([128, 2], F32, name="bq_sb")
    g.bk_sb = singles.tile([128, 2], F32, name="bk_sb")
    g.bvb_sb = singles.tile([128, DLOC], F32, name="bvb_sb")
    g.mneg_sb = singles.tile([128, NKC], F32, name="mneg_sb")
    nc.sync.dma_start(out=g.bq_sb, in_=ins["bq"].ap().rearrange("i p -> p i"))
    nc.scalar.dma_start(out=g.bk_sb, in_=ins["bk"].ap().rearrange("i p -> p i"))
    nc.sync.dma_start(out=g.bvb_sb, in_=ins["bv"].ap().partition_broadcast(128))
    nc.scalar.dma_start(out=g.mneg_sb, in_=ins["mneg"].ap())
    for cc in range(8):
        q_sx[cc % 2].dma_start(
            out=g.x_sb[cc][:, 512:1024], in_=xT_r[:, cc, 512:1024]
        )
    for cc in range(8):
        q_sx[cc % 2].dma_start(out=g.x_sb[cc][:, 1024:T], in_=xT_r[:, cc, 1024:T])

    # --- persistent activations -----------------------------------------
    g.qT_sb = singles.tile([128, 2, T], BF16, tag="qT", name="qT_sb")
    g.kT_sb = singles.tile([128, 2, T], BF16, tag="kT", name="kT_sb")
    g.vaug = [
        singles.tile([128, NKC, VAUG_W], BF16, tag=f"vaug{h}", name=f"vaug{h}")
        for h in range(4)
    ]
    for h in range(4):
        # col 64 = 1.0 -> psum row 64 = softmax denominator; cols 65:96 = 0
        # -> psum rows 65:96 = 0 (initialized, so the denominator transpose
        # reads no garbage)
        nc.vector.memset(g.vaug[h][:, :, D], 1.0)
        nc.vector.memset(g.vaug[h][:, :, D + 1 : VAUG_W], 0.0)
    g.yT_sb = singles.tile([128, 2, T], IN_DT, tag="yT", name="yT_sb")
    g.stg = {i: singles.tile([128, C], F32, name=f"stg{i}") for i in range(8, 16)}
    g.obs = {}

    # --- proj braid units -------------------------------------------------
    def ob_for(i):
        if i not in g.obs:
            g.obs[i] = g.outp.tile([128, C], OUT_DT, tag="ob", name=f"ob{i}")
        return g.obs[i]

    def ob_flush(i, queue):
        ob = g.obs.pop(i)
        queue.dma_start(out=out.ap()[i * 128 : (i + 1) * 128, :], in_=ob)

    def proj_full_half(i, hf):
        """tiles 0..7: both chunks of a 512-col output half -> ob fp16."""
        c0 = hf * 512
        psp = g.pool_x.tile([128, 512], F32, tag="px", name="psp")
        for step, ic in enumerate((1, 0)):
            nc.tensor.matmul(
                psp,
                g.yT_sb[:, ic, i * 128 : (i + 1) * 128],
                g.wp_sb[:, ic, c0 : c0 + 512],
                start=(step == 0),
                stop=(step == 1),
            )
        nc.vector.tensor_copy(ob_for(i)[:, c0 : c0 + 512], psp)
        if hf == 1:
            ob_flush(i, nc.gpsimd)

    def stage_half(i, hf):
        """tiles 8..15: chunk ic=1 partial -> fp32 stage."""
        c0 = hf * 512
        psp = g.pool_x.tile([128, 512], F32, tag="px", name="psp")
        nc.tensor.matmul(
            psp,
            g.yT_sb[:, 1, i * 128 : (i + 1) * 128],
            g.wp_sb[:, 1, c0 : c0 + 512],
            start=True,
            stop=True,
        )
        nc.vector.tensor_copy(g.stg[i][:, c0 : c0 + 512], psp)

    def finish_half(i, hf, eng, queue):
        """tiles 8..15: chunk ic=0 + staged ic=1 -> ob fp16."""
        c0 = hf * 512
        psp = g.pool_x.tile([128, 512], F32, tag="px", name="psp")
        nc.tensor.matmul(
            psp,
            g.yT_sb[:, 0, i * 128 : (i + 1) * 128],
            g.wp_sb[:, 0, c0 : c0 + 512],
            start=True,
            stop=True,
        )
        eng.tensor_tensor(
            out=ob_for(i)[:, c0 : c0 + 512],
            in0=psp,
            in1=g.stg[i][:, c0 : c0 + 512],
            op=AluOpType.add,
        )
        if hf == 1:
            ob_flush(i, queue)

    # --- schedule ---------------------------------------------------------
    _emit_qkv_tblock(nc, g, 0)
    _emit_qkv_tblock(nc, g, 1)
    _emit_attention_single(nc, g, 2, 0)
    _emit_qkv_tblock(nc, g, 2)
    _emit_attention_single(nc, g, 3, 0)
    _emit_qkv_tblock(nc, g, 3)
    # c_proj weights (sync queue is idle from here; needed by pair braids)
    g.wp_sb = singles.tile([128, 2, C], IN_DT, name="wp_sb")
    nc.sync.dma_start(
        out=g.wp_sb, in_=ins["wproj"].ap().rearrange("(i p) n -> p i n", p=128)
    )
    _emit_attention_pair(nc, g, 0, 1, 0)

    mk = lambda f, *a: (lambda: f(*a))
    units_07 = [mk(proj_full_half, i, hf) for i in range(0, 8) for hf in (0, 1)]
    units_stage = [mk(stage_half, i, hf) for i in range(8, 16) for hf in (0, 1)]
    units_fin_a = [
        mk(finish_half, i, hf, nc.vector, nc.gpsimd)
        for i in range(8, 12)
        for hf in (0, 1)
    ]
    _emit_attention_pair(nc, g, 2, 3, 1, braid=_spread(units_07, 4, 15))
    _emit_attention_pair(
        nc, g, 0, 1, 1,
        braid=_spread(units_stage, 4, 14) | _spread(units_fin_a, 16, 16),
    )
    # tail: finish the last four tiles (adds split DVE / gpsimd)
    for i in range(12, 16):
        for hf in (0, 1):
            finish_half(i, hf, nc.vector, nc.sync)


@functools.lru_cache(maxsize=1)
def _program():
    nc = bacc.Bacc("TRN2", target_bir_lowering=False, debug=False)
    shapes = {
        "xT": ([C, T], IN_DT),
        "wq": ([C, DLOC], IN_DT),
        "wk": ([C, DLOC], IN_DT),
        "wv": ([C, DLOC], IN_DT),
        "bq": ([2, 128], F32),
        "bk": ([2, 128], F32),
        "bv": ([1, DLOC], F32),
        "wproj": ([DLOC, C], IN_DT),
        "mneg": ([128, NKC], F32),
        "tri": ([128, 128], BF16),
    }
    ins = {
        name: nc.dram_tensor(name, shape, dt_, kind="ExternalInput")
        for name, (shape, dt_) in shapes.items()
    }
    out = nc.dram_tensor("out", [T, C], OUT_DT, kind="ExternalOutput")
    rn_dram = nc.dram_tensor("rn_scratch", [32, 512], F32, kind="Internal")
    with tile.TileContext(nc) as tc, contextlib.ExitStack() as ctx:
        _build(ctx, nc, tc, ins, out, rn_dram)
    nc.compile()
    return nc


def make_in_maps(x, attention_mask, W_attn, b_attn, W_proj, b_proj):
    import ml_dtypes

    in_np = ml_dtypes.bfloat16
    x = np.ascontiguousarray(np.asarray(x, dtype=np.float32))
    attention_mask = np.asarray(attention_mask, dtype=np.float32)
    W_attn = np.asarray(W_attn, dtype=np.float32)
    b_attn = np.asarray(b_attn, dtype=np.float32)
    W_proj = np.asarray(W_proj, dtype=np.float32)

    tri = (np.arange(128)[None, :] >= np.arange(128)[:, None]).astype(np.float32)
    in_maps = []
    for c in range(NCORES):
        b = c // 4
        g = c % 4
        cols = slice(g * DLOC, (g + 1) * DLOC)
        xT = np.ascontiguousarray(x[b].T.astype(in_np))
        mneg = np.ascontiguousarray((attention_mask[b] * NEG).reshape(NKC, 128).T)
        in_maps.append(
            {
                "xT": xT,
                "wq": np.ascontiguousarray(W_attn[:, cols].astype(in_np)),
                "wk": np.ascontiguousarray(W_attn[:, C : 2 * C][:, cols].astype(in_np)),
                "wv": np.ascontiguousarray(
                    W_attn[:, 2 * C : 3 * C][:, cols].astype(in_np)
                ),
                "bq": np.ascontiguousarray(b_attn[cols].reshape(2, 128)),
                "bk": np.ascontiguousarray(b_attn[C : 2 * C][cols].reshape(2, 128)),
                "bv": np.ascontiguousarray(b_attn[2 * C : 3 * C][cols].reshape(1, DLOC)),
                "wproj": np.ascontiguousarray(
                    W_proj[g * DLOC : (g + 1) * DLOC, :].astype(in_np)
                ),
                "mneg": mneg,
                "tri": tri.astype(in_np),
            }
        )
    return in_maps


def kernel(x, attention_mask, W_attn, b_attn, W_proj, b_proj, _res_hook=None):
    in_maps = make_in_maps(x, attention_mask, W_attn, b_attn, W_proj, b_proj)
    nc = _program()
    res = bass_utils.run_bass_kernel_spmd(nc, in_maps, core_ids=list(range(NCORES)))
    if _res_hook is not None:
        _res_hook(res)
    b_proj = np.asarray(b_proj, dtype=np.float32)
    y = np.zeros((B, T, C), dtype=np.float32)
    for c in range(NCORES):
        y[c // 4] += np.asarray(res.results[c]["out"], dtype=np.float32)
    y += b_proj[None, None, :]
    return y


# revision 16
# speedup vs baseline: 1.4918x; 1.4918x over previous
"""Causal self-attention (B=2, T=2048, C=1024, H=16) on 8 trn2 NeuronCores.

Sharding: batch x head-group. Core c handles batch b = c//4 and heads
[4*(c%4), 4*(c%4)+4). Each core computes qkv for its head slice, causal
attention, and a partial c_proj ([T, C] over its 256 input rows of W_proj);
the host sums the 4 fp16 partials per batch in fp32.

v4 schedule. Two co-critical resources: the ACT engine's exp stream (~80us:
1 elem/lane/cycle @1.2GHz over the 8.9M causal logits, irreducible) and the
PE's ~117us of bf16 matmul columns.  Design:
  - attention blocks run as two-head PAIRS with j-steps interleaved
    (S_A S_B | O_A O_B | braid): each block's S/O covers the partner's exp
    latency, keeping the PE stream dense so the HAM clock stays at 2.4GHz
    through the exp-paced back half;
  - softmax normalize without ACT and without slow reciprocals: the O
    accumulation carries [denom; zeros] in psum rows 64:96 (vaug columns
    64:96 = [1; 0...]), DVE-transposes them to put the 512 denominators
    across 32 partitions (reciprocal is ~6.5 cyc/elem on the free dim, so
    a [1,512] row costs 3.3us but the [32,16] spread costs 0.15us), then a
    dram round-trip broadcast feeds the [64,512] multiply; bank0 of each O
    group is final at j=8m+3, so its half normalizes four j-steps early;
  - c_proj is braided into the pairs as 512-col psum half-tiles: tiles 0..7
    both-chunk halves into pair (2,3|m=1), tiles 8..15 as chunk-1 stages
    into pair (0,1|m=1) plus chunk-0 finishes at the tail; output is fp16;
  - input x rides the sync/scalar queues in 512-col quarters (first mass =
    1MiB x + 2MiB weights at the ~270GB/s effective DMA roofline), weights
    rotate over sync/scalar/gpsimd; a ~7us junk burst covers the lead-in;
  - all psum->sbuf movement rides DVE (gpsimd has no PSUM port); the
    gpsimd queue carries the braided output DMAs, keeping ACT exp-only.
"""

import contextlib
import functools
import sys

sys.path.insert(0, "/opt/trn_rl_repo")

import numpy as np

import concourse.bacc as bacc
import concourse.mybir as mybir
import concourse.tile as tile
from concourse import bass_utils
from concourse import library_config
from concourse.alu_op_type import AluOpType

B, T, C, H, D = 2, 2048, 1024, 16, 64
NEG = -1e10
NCORES = 8
HEADS_PER_CORE = 4
DLOC = HEADS_PER_CORE * D  # 256 local head dims per core
F32 = mybir.dt.float32
F16 = mybir.dt.float16
BF16 = mybir.dt.bfloat16
AF = mybir.ActivationFunctionType

IN_DT = BF16
OUT_DT = F16
# ~duration of junk pre-warm matmuls covering the input-DMA lead-in (ns)
WARM_NS = 7000

NTB = T // 512  # 4 t-blocks in qkv phase
NKC = T // 128  # 16 k-chunks
VAUG_W = 96  # v cols 0:64, ones col 64, zero cols 65:96 (psum transposable)


def _pieces(a, end=1024):
    """Split [a, end) at 512-boundaries (psum bank boundaries)."""
    cuts = [a]
    b = (a // 512 + 1) * 512
    while b < end:
        cuts.append(b)
        b += 512
    cuts.append(end)
    return list(zip(cuts[:-1], cuts[1:]))


class Ctx:
    pass


def _emit_prewarm(nc, g):
    """Dependency-free fp32 matmuls covering the input-DMA lead-in so the
    PE hands off at full HAM clock to the dense qkv stream."""
    n = 3 + max(0, int((WARM_NS - 5200) / 880))
    ps = g.pool_x.tile([128, 1024], F32, tag="px", name="warm_ps")
    for i in range(n):
        nc.tensor.matmul(
            ps[:, 0:512],
            g.warm_sb[:, 0:128],
            g.warm_sb,
            start=(i == 0),
            stop=(i == n - 1),
        )
    wsink = g.rnpool.tile([1, 128], F32, tag="wsink", name="wsink", bufs=1)
    nc.vector.tensor_copy(wsink, ps[0:1, 0:128])
    nc.sync.dma_start(out=g.rn_dram.ap()[31:32, 0:128], in_=wsink)


def _emit_qkv_tblock(nc, g, tb):
    """qkv projections for t in [tb*512, (tb+1)*512), chunk-outer.

    v chains for ts 2,3 (psvB) ride the pool_o slot of the PREVIOUS
    attention block's pso, which frees only after its normalize chain:
    they run in a second pass so the PE reaches them ~5us in.
    """
    psq = g.pool_x.tile([128, 1024], F32, tag="px", name="psq")
    psk = g.pool_x.tile([128, 1024], F32, tag="px", name="psk")
    psv = [
        g.pool_o.tile([128, 1024], F32, tag="po", name="psvA"),
        g.pool_o.tile([128, 1024], F32, tag="po", name="psvB"),
    ]

    def vslice(ts, width=256):
        return psv[ts // 2][:, (ts % 2) * 512 : (ts % 2) * 512 + width]

    for cc in range(8):
        xts = g.x_sb[cc][:, tb * 512 : (tb + 1) * 512]
        st, sp = cc == 0, cc == 7
        for dt_ in range(2):
            nc.tensor.matmul(
                psq[:, dt_ * 512 : (dt_ + 1) * 512],
                g.wq_sb[cc][:, dt_ * 128 : (dt_ + 1) * 128],
                xts,
                start=st,
                stop=sp,
            )
            nc.tensor.matmul(
                psk[:, dt_ * 512 : (dt_ + 1) * 512],
                g.wk_sb[cc][:, dt_ * 128 : (dt_ + 1) * 128],
                xts,
                start=st,
                stop=sp,
            )
        for ts in range(2):
            nc.tensor.matmul(
                vslice(ts),
                xts[:, ts * 128 : (ts + 1) * 128],
                g.wv_sb[cc],
                start=st,
                stop=sp,
            )
    for cc in range(8):
        xts = g.x_sb[cc][:, tb * 512 : (tb + 1) * 512]
        st, sp = cc == 0, cc == 7
        for ts in range(2, 4):
            nc.tensor.matmul(
                vslice(ts),
                xts[:, ts * 128 : (ts + 1) * 128],
                g.wv_sb[cc],
                start=st,
                stop=sp,
            )
    for dt_ in range(2):
        nc.vector.tensor_scalar(
            out=g.qT_sb[:, dt_, tb * 512 : (tb + 1) * 512],
            in0=psq[:, dt_ * 512 : (dt_ + 1) * 512],
            scalar1=g.bq_sb[:, dt_ : dt_ + 1],
            scalar2=None,
            op0=AluOpType.add,
        )
        nc.vector.tensor_scalar(
            out=g.kT_sb[:, dt_, tb * 512 : (tb + 1) * 512],
            in0=psk[:, dt_ * 512 : (dt_ + 1) * 512],
            scalar1=g.bk_sb[:, dt_ : dt_ + 1],
            scalar2=None,
            op0=AluOpType.add,
        )
    for ts in range(4):
        kc = tb * 4 + ts
        for h in range(4):
            nc.vector.tensor_tensor(
                out=g.vaug[h][:, kc, 0:D],
                in0=vslice(ts)[:, h * D : (h + 1) * D],
                in1=g.bvb_sb[:, h * D : (h + 1) * D],
                op=AluOpType.add,
            )


class Block:
    """Emission state for one head x one 1024-wide q-block of attention."""

    def __init__(self, nc, g, h, m):
        self.nc, self.g, self.h, self.m = nc, g, h, m
        self.prow = (h % 2) * 64
        self.pi = h // 2
        self.njs = 8 * m + 8
        self.last_b0 = 8 * m + 3
        self.pso = g.pool_o.tile([128, 1024], F32, tag="po", name="pso")
        self.uts = {}

    def S_exp(self, j):
        nc, g = self.nc, self.g
        a = max(0, 128 * j - 1024 * self.m)
        pss = g.pool_x.tile([128, 1024], F32, tag="px", name="pss")
        for c0, c1 in _pieces(a):
            nc.tensor.matmul(
                pss[:, c0:c1],
                g.kT_sb[self.prow : self.prow + 64, self.pi, j * 128 : (j + 1) * 128],
                g.qT_sb[
                    self.prow : self.prow + 64,
                    self.pi,
                    self.m * 1024 + c0 : self.m * 1024 + c1,
                ],
                start=True,
                stop=True,
            )
        ut = g.utpool.tile([128, 1024], BF16, tag="ut", name="ut")
        self.uts[j] = ut
        nc.scalar.activation(
            out=ut[:, a:1024],
            in_=pss[:, a:1024],
            func=AF.Exp,
            bias=g.mneg_sb[:, j : j + 1],
            scale=0.125,
        )
        if j >= 8 * self.m:
            nc.vector.tensor_mul(ut[:, a : a + 128], ut[:, a : a + 128], g.tri_sb)

    def O(self, j):
        nc, g = self.nc, self.g
        a = max(0, 128 * j - 1024 * self.m)
        ut = self.uts.pop(j)
        for c0, c1 in _pieces(a):
            stop = j == (self.last_b0 if c0 < 512 else self.njs - 1)
            nc.tensor.matmul(
                self.pso[0:VAUG_W, c0:c1],
                g.vaug[self.h][:, j, :],
                ut[:, c0:c1],
                start=(j == 0),
                stop=stop,
            )

    def chain(self, half):
        """Start the 1/denom pipeline for psum bank `half` (cols 512*half+).

        psum rows 64:96 hold [denom; zeros]; a DVE 32x32-block transpose
        spreads the 512 denominators over 32 partitions (16 per lane, at
        free-offsets 0,32,...) so the ~6.5 cyc/elem reciprocal costs 0.15us
        instead of 3.3; a second transpose gathers the recips back into a
        q-ordered row, and gpsimd broadcasts it to [64, 512].  All engine
        ops - no DMA round trips (a strided dram hop costs 512 descriptors)."""
        nc, g = self.nc, self.g
        c0 = half * 512
        dT = g.rnpool.tile([32, 512], F32, tag="dT", name="dT")
        nc.vector.transpose(dT, self.pso[64:96, c0 : c0 + 512])
        dT_s = dT.rearrange("p (b q) -> p b q", q=32)[:, :, 0]
        nc.vector.reciprocal(dT_s, dT_s)
        rrow = g.rnpool.tile([32, 512], F32, tag="rrow", name="rrow")
        nc.vector.transpose(rrow, dT)
        rnb = g.rnpool.tile([64, 512], F32, tag="rnb", name="rnb")
        nc.gpsimd.partition_broadcast(rnb, rrow[0:1, :])
        self._rnb = getattr(self, "_rnb", {})
        self._rnb[half] = rnb

    def mult(self, half):
        nc, g = self.nc, self.g
        c0 = half * 512
        nc.vector.tensor_tensor(
            out=g.yT_sb[
                self.prow : self.prow + 64,
                self.pi,
                self.m * 1024 + c0 : self.m * 1024 + c0 + 512,
            ],
            in0=self.pso[0:D, c0 : c0 + 512],
            in1=self._rnb.pop(half),
            op=AluOpType.mult,
        )


def _emit_attention_single(nc, g, h, m, braid=None):
    """One block alone (braided externally with qkv at the sequence level).
    bank0 chain starts at j=last_b0+1, its mult 3 steps later; bank1 at
    the end (the DVE wait hides behind the following qkv's bias lag)."""
    blk = Block(nc, g, h, m)
    braid = braid or {}
    blk.S_exp(0)
    for j in range(1, blk.njs):
        blk.S_exp(j)
        blk.O(j - 1)
        if j - 1 == blk.last_b0:
            blk.chain(0)
        if j == blk.last_b0 + 4:
            blk.mult(0)
        for fn in braid.get(j, ()):
            fn()
    blk.O(blk.njs - 1)
    blk.chain(1)
    blk.mult(1)
    for fn in braid.get(blk.njs, ()):
        fn()


def _emit_attention_pair(nc, g, hA, hB, m, braid=None):
    """Two heads' blocks with interleaved j-steps: each block's S/O covers
    the partner's exp latency so the PE stream stays dense."""
    A = Block(nc, g, hA, m)
    Bb = Block(nc, g, hB, m)
    braid = braid or {}
    A.S_exp(0)
    Bb.S_exp(0)
    for j in range(1, A.njs):
        A.S_exp(j)
        Bb.S_exp(j)
        A.O(j - 1)
        Bb.O(j - 1)
        if j - 1 == A.last_b0:
            A.chain(0)
            Bb.chain(0)
        if j == A.last_b0 + 4:
            A.mult(0)
            Bb.mult(0)
        for fn in braid.get(j, ()):
            fn()
    A.O(A.njs - 1)
    Bb.O(Bb.njs - 1)
    A.chain(1)
    Bb.chain(1)
    A.mult(1)
    Bb.mult(1)
    for fn in braid.get(A.njs, ()):
        fn()


def _spread(items, j0, j1):
    """Distribute items over j positions [j0, j1] -> dict j -> [item]."""
    out = {}
    n = len(items)
    span = j1 - j0 + 1
    for idx, it in enumerate(items):
        j = j0 + (idx * span) // n
        out.setdefault(j, []).append(it)
    return out


def _build(ctx, nc, tc, ins, out, rn_dram):
    g = Ctx()
    g.rn_dram = rn_dram

    singles = ctx.enter_context(tc.tile_pool(name="singles", bufs=1))
    g.pool_x = ctx.enter_context(tc.tile_pool(name="pool_x", bufs=2, space="PSUM"))
    g.pool_o = ctx.enter_context(tc.tile_pool(name="pool_o", bufs=2, space="PSUM"))
    g.utpool = ctx.enter_context(tc.tile_pool(name="utpool", bufs=6))
    g.rnpool = ctx.enter_context(tc.tile_pool(name="rnpool", bufs=2))
    g.outp = ctx.enter_context(tc.tile_pool(name="outp", bufs=3))

    nc.gpsimd.load_library(library_config.attn)

    # tri mask first (sync queue): the pre-warm burst depends only on memset
    g.tri_sb = singles.tile([128, 128], BF16, name="tri_sb")
    nc.sync.dma_start(out=g.tri_sb, in_=ins["tri"].ap())
    g.warm_sb = singles.tile([128, 512], F32, name="warm_sb")
    nc.vector.memset(g.warm_sb, 0.5)
    _emit_prewarm(nc, g)

    # --- inputs -----------------------------------------------------------
    # x quarters for tblocks 0,1 land first (the ~3MiB first-mass rides the
    # ~270GB/s effective DMA roofline); the t>=1024 halves follow. Weights
    # rotate over all three DMA-capable queues.
    q_sx = [nc.sync, nc.scalar]
    q3 = [nc.sync, nc.scalar, nc.gpsimd]

    g.x_sb = [singles.tile([128, T], IN_DT, name=f"x{c}") for c in range(8)]
    g.wq_sb = [singles.tile([128, DLOC], IN_DT, name=f"wq{c}") for c in range(8)]
    g.wk_sb = [singles.tile([128, DLOC], IN_DT, name=f"wk{c}") for c in range(8)]
    g.wv_sb = [singles.tile([128, DLOC], IN_DT, name=f"wv{c}") for c in range(8)]
    xT_r = ins["xT"].ap().rearrange("(c p) t -> p c t", p=128)
    wq_src = ins["wq"].ap().rearrange("(c p) m -> p c m", p=128)
    wk_src = ins["wk"].ap().rearrange("(c p) m -> p c m", p=128)
    wv_src = ins["wv"].ap().rearrange("(c p) m -> p c m", p=128)
    for cc in range(8):
        q_sx[cc % 2].dma_start(out=g.x_sb[cc][:, 0:512], in_=xT_r[:, cc, 0:512])
        q3[(cc + 0) % 3].dma_start(out=g.wq_sb[cc], in_=wq_src[:, cc, :])
        q3[(cc + 1) % 3].dma_start(out=g.wk_sb[cc], in_=wk_src[:, cc, :])
        q3[(cc + 2) % 3].dma_start(out=g.wv_sb[cc], in_=wv_src[:, cc, :])

    g.bq_sb = singles.tile([128, 2], F32, name="bq_sb")
    g.bk_sb = singles.tile([128, 2], F32, name="bk_sb")
    g.bvb_sb = singles.tile([128, DLOC], F32, name="bvb_sb")
    g.mneg_sb = singles.tile([128, NKC], F32, name="mneg_sb")
    nc.sync.dma_start(out=g.bq_sb, in_=ins["bq"].ap().rearrange("i p -> p i"))
    nc.scalar.dma_start(out=g.bk_sb, in_=ins["bk"].ap().rearrange("i p -> p i"))
    nc.sync.dma_start(out=g.bvb_sb, in_=ins["bv"].ap().partition_broadcast(128))
    nc.scalar.dma_start(out=g.mneg_sb, in_=ins["mneg"].ap())
    for cc in range(8):
        q_sx[cc % 2].dma_start(
            out=g.x_sb[cc][:, 512:1024], in_=xT_r[:, cc, 512:1024]
        )
    for cc in range(8):
        q_sx[cc % 2].dma_start(out=g.x_sb[cc][:, 1024:T], in_=xT_r[:, cc, 1024:T])

    # --- persistent activations -----------------------------------------
    g.qT_sb = singles.tile([128, 2, T], BF16, tag="qT", name="qT_sb")
    g.kT_sb = singles.tile([128, 2, T], BF16, tag="kT", name="kT_sb")
    g.vaug = [
        singles.tile([128, NKC, VAUG_W], BF16, tag=f"vaug{h}", name=f"vaug{h}")
        for h in range(4)
    ]
    for h in range(4):
        # col 64 = 1.0 -> psum row 64 = softmax denominator; cols 65:96 = 0
        # -> psum rows 65:96 = 0 (initialized, so the denominator transpose
        # reads no garbage)
        nc.vector.memset(g.vaug[h][:, :, D], 1.0)
        nc.vector.memset(g.vaug[h][:, :, D + 1 : VAUG_W], 0.0)
    g.yT_sb = singles.tile([128, 2, T], IN_DT, tag="yT", name="yT_sb")
    g.stg = {i: singles.tile([128, C], F32, name=f"stg{i}") for i in range(8, 16)}
    g.obs = {}

    # --- proj braid units -------------------------------------------------
    def ob_for(i):
        if i not in g.obs:
            g.obs[i] = g.outp.tile([128, C], OUT_DT, tag="ob", name=f"ob{i}")
        return g.obs[i]

    def ob_flush(i, queue):
        ob = g.obs.pop(i)
        queue.dma_start(out=out.ap()[i * 128 : (i + 1) * 128, :], in_=ob)

    def proj_full_half(i, hf):
        """tiles 0..7: both chunks of a 512-col output half -> ob fp16."""
        c0 = hf * 512
        psp = g.pool_x.tile([128, 512], F32, tag="px", name="psp")
        for step, ic in enumerate((1, 0)):
            nc.tensor.matmul(
                psp,
                g.yT_sb[:, ic, i * 128 : (i + 1) * 128],
                g.wp_sb[:, ic, c0 : c0 + 512],
                start=(step == 0),
                stop=(step == 1),
            )
        nc.vector.tensor_copy(ob_for(i)[:, c0 : c0 + 512], psp)
        if hf == 1:
            ob_flush(i, nc.gpsimd)

    def stage_half(i, hf):
        """tiles 8..15: chunk ic=1 partial -> fp32 stage."""
        c0 = hf * 512
        psp = g.pool_x.tile([128, 512], F32, tag="px", name="psp")
        nc.tensor.matmul(
            psp,
            g.yT_sb[:, 1, i * 128 : (i + 1) * 128],
            g.wp_sb[:, 1, c0 : c0 + 512],
            start=True,
            stop=True,
        )
        nc.vector.tensor_copy(g.stg[i][:, c0 : c0 + 512], psp)

    def finish_half(i, hf, eng, queue):
        """tiles 8..15: chunk ic=0 + staged ic=1 -> ob fp16."""
        c0 = hf * 512
        psp = g.pool_x.tile([128, 512], F32, tag="px", name="psp")
        nc.tensor.matmul(
            psp,
            g.yT_sb[:, 0, i * 128 : (i + 1) * 128],
            g.wp_sb[:, 0, c0 : c0 + 512],
            start=True,
            stop=True,
        )
        eng.tensor_tensor(
            out=ob_for(i)[:, c0 : c0 + 512],
            in0=psp,
            in1=g.stg[i][:, c0 : c0 + 512],
            op=AluOpType.add,
        )
        if hf == 1:
            ob_flush(i, queue)

    # --- schedule ---------------------------------------------------------
    _emit_qkv_tblock(nc, g, 0)
    _emit_qkv_tblock(nc, g, 1)
    _emit_attention_single(nc, g, 2, 0)
    _emit_qkv_tblock(nc, g, 2)
    _emit_attention_single(nc, g, 3, 0)
    _emit_qkv_tblock(nc, g, 3)
    # c_proj weights (sync queue is idle from here; needed by pair braids)
    g.wp_sb = singles.tile([128, 2, C], IN_DT, name="wp_sb")
    nc.sync.dma_start(
        out=g.wp_sb, in_=ins["wproj"].ap().rearrange("(i p) n -> p i n", p=128)
    )
    _emit_attention_pair(nc, g, 0, 1, 0)

    mk = lambda f, *a: (lambda: f(*a))
    units_07 = [mk(proj_full_half, i, hf) for i in range(0, 8) for hf in (0, 1)]
    units_stage = [mk(stage_half, i, hf) for i in range(8, 16) for hf in (0, 1)]
    units_fin_a = [
        mk(finish_half, i, hf, nc.vector, nc.gpsimd)
        for i in range(8, 12)
        for hf in (0, 1)
    ]
    _emit_attention_pair(nc, g, 2, 3, 1, braid=_spread(units_07, 4, 15))
    _emit_attention_pair(
        nc, g, 0, 1, 1,
        braid=_spread(units_stage, 4, 14) | _spread(units_fin_a, 16, 16),
    )
    # tail: finish the last four tiles (adds split DVE / gpsimd)
    for i in range(12, 16):
        for hf in (0, 1):
            finish_half(i, hf, nc.vector, nc.sync)


@functools.lru_cache(maxsize=1)
def _program():
    nc = bacc.Bacc("TRN2", target_bir_lowering=False, debug=False)
    shapes = {
        "xT": ([C, T], IN_DT),
        "wq": ([C, DLOC], IN_DT),
        "wk": ([C, DLOC], IN_DT),
        "wv": ([C, DLOC], IN_DT),
        "bq": ([2, 128], F32),
        "bk": ([2, 128], F32),
        "bv": ([1, DLOC], F32),
        "wproj": ([DLOC, C], IN_DT),
        "mneg": ([128, NKC], F32),
        "tri": ([128, 128], BF16),
    }
    ins = {
        name: nc.dram_tensor(name, shape, dt_, kind="ExternalInput")
        for name, (shape, dt_) in shapes.items()
    }
    out = nc.dram_tensor("out", [T, C], OUT_DT, kind="ExternalOutput")
    rn_dram = nc.dram_tensor("rn_scratch", [32, 512], F32, kind="Internal")
    with tile.TileContext(nc) as tc, contextlib.ExitStack() as ctx:
        _build(ctx, nc, tc, ins, out, rn_dram)
    nc.compile()
    return nc


def make_in_maps(x, attention_mask, W_attn, b_attn, W_proj, b_proj):
    import ml_dtypes

    in_np = ml_dtypes.bfloat16
    x = np.ascontiguousarray(np.asarray(x, dtype=np.float32))
    attention_mask = np.asarray(attention_mask, dtype=np.float32)
    W_attn = np.asarray(W_attn, dtype=np.float32)
    b_attn = np.asarray(b_attn, dtype=np.float32)
    W_proj = np.asarray(W_proj, dtype=np.float32)

    tri = (np.arange(128)[None, :] >= np.arange(128)[:, None]).astype(np.float32)
    in_maps = []
    for c in range(NCORES):
        b = c // 4
        g = c % 4
        cols = slice(g * DLOC, (g + 1) * DLOC)
        xT = np.ascontiguousarray(x[b].T.astype(in_np))
        mneg = np.ascontiguousarray((attention_mask[b] * NEG).reshape(NKC, 128).T)
        in_maps.append(
            {
                "xT": xT,
                "wq": np.ascontiguousarray(W_attn[:, cols].astype(in_np)),
                "wk": np.ascontiguousarray(W_attn[:, C : 2 * C][:, cols].astype(in_np)),
                "wv": np.ascontiguousarray(
                    W_attn[:, 2 * C : 3 * C][:, cols].astype(in_np)
                ),
                "bq": np.ascontiguousarray(b_attn[cols].reshape(2, 128)),
                "bk": np.ascontiguousarray(b_attn[C : 2 * C][cols].reshape(2, 128)),
                "bv": np.ascontiguousarray(b_attn[2 * C : 3 * C][cols].reshape(1, DLOC)),
                "wproj": np.ascontiguousarray(
                    W_proj[g * DLOC : (g + 1) * DLOC, :].astype(in_np)
                ),
                "mneg": mneg,
                "tri": tri.astype(in_np),
            }
        )
    return in_maps


def kernel(x, attention_mask, W_attn, b_attn, W_proj, b_proj, _res_hook=None):
    in_maps = make_in_maps(x, attention_mask, W_attn, b_attn, W_proj, b_proj)
    nc = _program()
    res = bass_utils.run_bass_kernel_spmd(nc, in_maps, core_ids=list(range(NCORES)))
    if _res_hook is not None:
        _res_hook(res)
    b_proj = np.asarray(b_proj, dtype=np.float32)
    y = np.zeros((B, T, C), dtype=np.float32)
    for c in range(NCORES):
        y[c // 4] += np.asarray(res.results[c]["out"], dtype=np.float32)
    y += b_proj[None, None, :]
    return y


# revision 19
# speedup vs baseline: 1.5126x; 1.0140x over previous
"""Causal self-attention (B=2, T=2048, C=1024, H=16) on 8 trn2 NeuronCores.

Sharding: batch x head-group. Core c handles batch b = c//4 and heads
[4*(c%4), 4*(c%4)+4). Each core computes qkv for its head slice, causal
attention, and a partial c_proj ([T, C] over its 256 input rows of W_proj);
the host sums the 4 fp16 partials per batch in fp32.

v4 schedule. Two co-critical resources: the ACT engine's exp stream (~80us:
1 elem/lane/cycle @1.2GHz over the 8.9M causal logits, irreducible) and the
PE's ~117us of bf16 matmul columns.  Design:
  - attention blocks run as two-head PAIRS with j-steps interleaved
    (S_A S_B | O_A O_B | braid): each block's S/O covers the partner's exp
    latency, keeping the PE stream dense so the HAM clock stays at 2.4GHz
    through the exp-paced back half;
  - softmax normalize without ACT and without slow reciprocals: the O
    accumulation carries [denom; zeros] in psum rows 64:96 (vaug columns
    64:96 = [1; 0...]), DVE-transposes them to put the 512 denominators
    across 32 partitions (reciprocal is ~6.5 cyc/elem on the free dim, so
    a [1,512] row costs 3.3us but the [32,16] spread costs 0.15us), then a
    dram round-trip broadcast feeds the [64,512] multiply; bank0 of each O
    group is final at j=8m+3, so its half normalizes four j-steps early;
  - c_proj is braided into the pairs as 512-col psum half-tiles: tiles 0..7
    both-chunk halves into pair (2,3|m=1), tiles 8..15 as chunk-1 stages
    into pair (0,1|m=1) plus chunk-0 finishes at the tail; output is fp16;
  - input x rides the sync/scalar queues in 512-col quarters (first mass =
    1MiB x + 2MiB weights at the ~270GB/s effective DMA roofline), weights
    rotate over sync/scalar/gpsimd; a ~7us junk burst covers the lead-in;
  - all psum->sbuf movement rides DVE (gpsimd has no PSUM port); the
    gpsimd queue carries the braided output DMAs, keeping ACT exp-only.
"""

import contextlib
import functools
import sys

sys.path.insert(0, "/opt/trn_rl_repo")

import numpy as np

import concourse.bacc as bacc
import concourse.mybir as mybir
import concourse.tile as tile
from concourse import bass_utils
from concourse import library_config
from concourse.alu_op_type import AluOpType

B, T, C, H, D = 2, 2048, 1024, 16, 64
NEG = -1e10
NCORES = 8
HEADS_PER_CORE = 4
DLOC = HEADS_PER_CORE * D  # 256 local head dims per core
F32 = mybir.dt.float32
F16 = mybir.dt.float16
BF16 = mybir.dt.bfloat16
AF = mybir.ActivationFunctionType

IN_DT = BF16
OUT_DT = F16
# ~duration of junk pre-warm matmuls covering the input-DMA lead-in (ns)
WARM_NS = 9500

NTB = T // 512  # 4 t-blocks in qkv phase
NKC = T // 128  # 16 k-chunks
VAUG_W = 96  # v cols 0:64, ones col 64, zero cols 65:96 (psum transposable)


def _pieces(a, end=1024):
    """Split [a, end) at 512-boundaries (psum bank boundaries)."""
    cuts = [a]
    b = (a // 512 + 1) * 512
    while b < end:
        cuts.append(b)
        b += 512
    cuts.append(end)
    return list(zip(cuts[:-1], cuts[1:]))


class Ctx:
    pass


def _emit_prewarm(nc, g):
    """Dependency-free fp32 matmuls covering the input-DMA lead-in so the
    PE hands off at full HAM clock to the dense qkv stream."""
    n = 3 + max(0, int((WARM_NS - 5200) / 880))
    ps = g.pool_x.tile([128, 1024], F32, tag="px", name="warm_ps")
    for i in range(n):
        nc.tensor.matmul(
            ps[:, 0:512],
            g.warm_sb[:, 0:128],
            g.warm_sb,
            start=(i == 0),
            stop=(i == n - 1),
        )
    wsink = g.rnpool.tile([1, 128], F32, tag="wsink", name="wsink", bufs=1)
    nc.vector.tensor_copy(wsink, ps[0:1, 0:128])
    nc.sync.dma_start(out=g.rn_dram.ap()[31:32, 0:128], in_=wsink)


def _emit_qkv_tblock(nc, g, tb):
    """qkv projections for t in [tb*512, (tb+1)*512), chunk-outer.

    v chains for ts 2,3 (psvB) ride the pool_o slot of the PREVIOUS
    attention block's pso, which frees only after its normalize chain:
    they run in a second pass so the PE reaches them ~5us in.
    """
    psq = g.pool_x.tile([128, 1024], F32, tag="px", name="psq")
    psk = g.pool_x.tile([128, 1024], F32, tag="px", name="psk")
    psv = [
        g.pool_o.tile([128, 1024], F32, tag="po", name="psvA"),
        g.pool_o.tile([128, 1024], F32, tag="po", name="psvB"),
    ]

    def vslice(ts, width=256):
        return psv[ts // 2][:, (ts % 2) * 512 : (ts % 2) * 512 + width]

    for cc in range(8):
        xts = g.x_sb[cc][:, tb * 512 : (tb + 1) * 512]
        st, sp = cc == 0, cc == 7
        for dt_ in range(2):
            nc.tensor.matmul(
                psq[:, dt_ * 512 : (dt_ + 1) * 512],
                g.wq_sb[cc][:, dt_ * 128 : (dt_ + 1) * 128],
                xts,
                start=st,
                stop=sp,
            )
            nc.tensor.matmul(
                psk[:, dt_ * 512 : (dt_ + 1) * 512],
                g.wk_sb[cc][:, dt_ * 128 : (dt_ + 1) * 128],
                xts,
                start=st,
                stop=sp,
            )
        for ts in range(2):
            nc.tensor.matmul(
                vslice(ts),
                xts[:, ts * 128 : (ts + 1) * 128],
                g.wv_sb[cc],
                start=st,
                stop=sp,
            )
    for cc in range(8):
        xts = g.x_sb[cc][:, tb * 512 : (tb + 1) * 512]
        st, sp = cc == 0, cc == 7
        for ts in range(2, 4):
            nc.tensor.matmul(
                vslice(ts),
                xts[:, ts * 128 : (ts + 1) * 128],
                g.wv_sb[cc],
                start=st,
                stop=sp,
            )
    for dt_ in range(2):
        nc.vector.tensor_scalar(
            out=g.qT_sb[:, dt_, tb * 512 : (tb + 1) * 512],
            in0=psq[:, dt_ * 512 : (dt_ + 1) * 512],
            scalar1=g.bq_sb[:, dt_ : dt_ + 1],
            scalar2=None,
            op0=AluOpType.add,
        )
        nc.vector.tensor_scalar(
            out=g.kT_sb[:, dt_, tb * 512 : (tb + 1) * 512],
            in0=psk[:, dt_ * 512 : (dt_ + 1) * 512],
            scalar1=g.bk_sb[:, dt_ : dt_ + 1],
            scalar2=None,
            op0=AluOpType.add,
        )
    for ts in range(4):
        kc = tb * 4 + ts
        for h in range(4):
            nc.vector.tensor_tensor(
                out=g.vaug[h][:, kc, 0:D],
                in0=vslice(ts)[:, h * D : (h + 1) * D],
                in1=g.bvb_sb[:, h * D : (h + 1) * D],
                op=AluOpType.add,
            )


class Block:
    """Emission state for one head x one 1024-wide q-block of attention."""

    def __init__(self, nc, g, h, m):
        self.nc, self.g, self.h, self.m = nc, g, h, m
        self.prow = (h % 2) * 64
        self.pi = h // 2
        self.njs = 8 * m + 8
        self.last_b0 = 8 * m + 3
        self.pso = g.pool_o.tile([128, 1024], F32, tag="po", name="pso")
        self.uts = {}

    def S_exp(self, j):
        nc, g = self.nc, self.g
        a = max(0, 128 * j - 1024 * self.m)
        pss = g.pool_x.tile([128, 1024], F32, tag="px", name="pss")
        for c0, c1 in _pieces(a):
            nc.tensor.matmul(
                pss[:, c0:c1],
                g.kT_sb[self.prow : self.prow + 64, self.pi, j * 128 : (j + 1) * 128],
                g.qT_sb[
                    self.prow : self.prow + 64,
                    self.pi,
                    self.m * 1024 + c0 : self.m * 1024 + c1,
                ],
                start=True,
                stop=True,
            )
        ut = g.utpool.tile([128, 1024], BF16, tag="ut", name="ut")
        self.uts[j] = ut
        nc.scalar.activation(
            out=ut[:, a:1024],
            in_=pss[:, a:1024],
            func=AF.Exp,
            bias=g.mneg_sb[:, j : j + 1],
            scale=0.125,
        )
        if j >= 8 * self.m:
            nc.vector.tensor_mul(ut[:, a : a + 128], ut[:, a : a + 128], g.tri_sb)

    def O(self, j):
        nc, g = self.nc, self.g
        a = max(0, 128 * j - 1024 * self.m)
        ut = self.uts.pop(j)
        for c0, c1 in _pieces(a):
            stop = j == (self.last_b0 if c0 < 512 else self.njs - 1)
            nc.tensor.matmul(
                self.pso[0:VAUG_W, c0:c1],
                g.vaug[self.h][:, j, :],
                ut[:, c0:c1],
                start=(j == 0),
                stop=stop,
            )

    def chain(self, half):
        """Start the 1/denom pipeline for psum bank `half` (cols 512*half+).

        psum rows 64:96 hold [denom; zeros]; a DVE 32x32-block transpose
        spreads the 512 denominators over 32 partitions (16 per lane, at
        free-offsets 0,32,...) so the ~6.5 cyc/elem reciprocal costs 0.15us
        instead of 3.3; a second transpose gathers the recips back into a
        q-ordered row, and gpsimd broadcasts it to [64, 512].  All engine
        ops - no DMA round trips (a strided dram hop costs 512 descriptors)."""
        nc, g = self.nc, self.g
        c0 = half * 512
        dT = g.rnpool.tile([32, 512], F32, tag="dT", name="dT")
        nc.vector.transpose(dT, self.pso[64:96, c0 : c0 + 512])
        dT_s = dT.rearrange("p (b q) -> p b q", q=32)[:, :, 0]
        nc.vector.reciprocal(dT_s, dT_s)
        rrow = g.rnpool.tile([32, 512], F32, tag="rrow", name="rrow")
        nc.vector.transpose(rrow, dT)
        rnb = g.rnpool.tile([64, 512], F32, tag="rnb", name="rnb")
        nc.gpsimd.partition_broadcast(rnb, rrow[0:1, :])
        self._rnb = getattr(self, "_rnb", {})
        self._rnb[half] = rnb

    def mult(self, half):
        nc, g = self.nc, self.g
        c0 = half * 512
        nc.vector.tensor_tensor(
            out=g.yT_sb[
                self.prow : self.prow + 64,
                self.pi,
                self.m * 1024 + c0 : self.m * 1024 + c0 + 512,
            ],
            in0=self.pso[0:D, c0 : c0 + 512],
            in1=self._rnb.pop(half),
            op=AluOpType.mult,
        )


def _emit_attention_single(nc, g, h, m, braid=None):
    """One block alone (braided externally with qkv at the sequence level).
    bank0 chain starts at j=last_b0+1, its mult 3 steps later; bank1 at
    the end (the DVE wait hides behind the following qkv's bias lag)."""
    blk = Block(nc, g, h, m)
    braid = braid or {}
    blk.S_exp(0)
    for j in range(1, blk.njs):
        blk.S_exp(j)
        blk.O(j - 1)
        if j - 1 == blk.last_b0:
            blk.chain(0)
        if j == blk.last_b0 + 4:
            blk.mult(0)
        for fn in braid.get(j, ()):
            fn()
    blk.O(blk.njs - 1)
    blk.chain(1)
    blk.mult(1)
    for fn in braid.get(blk.njs, ()):
        fn()


def _emit_attention_pair(nc, g, hA, hB, m, braid=None):
    """Two heads' blocks with interleaved j-steps: each block's S/O covers
    the partner's exp latency so the PE stream stays dense."""
    A = Block(nc, g, hA, m)
    Bb = Block(nc, g, hB, m)
    braid = braid or {}
    A.S_exp(0)
    Bb.S_exp(0)
    for j in range(1, A.njs):
        A.S_exp(j)
        Bb.S_exp(j)
        A.O(j - 1)
        Bb.O(j - 1)
        if j - 1 == A.last_b0:
            A.chain(0)
            Bb.chain(0)
        if j == A.last_b0 + 4:
            A.mult(0)
            Bb.mult(0)
        for fn in braid.get(j, ()):
            fn()
    A.O(A.njs - 1)
    Bb.O(Bb.njs - 1)
    A.chain(1)
    Bb.chain(1)
    A.mult(1)
    Bb.mult(1)
    for fn in braid.get(A.njs, ()):
        fn()


def _spread(items, j0, j1):
    """Distribute items over j positions [j0, j1] -> dict j -> [item]."""
    out = {}
    n = len(items)
    span = j1 - j0 + 1
    for idx, it in enumerate(items):
        j = j0 + (idx * span) // n
        out.setdefault(j, []).append(it)
    return out


def _build(ctx, nc, tc, ins, out, rn_dram):
    g = Ctx()
    g.rn_dram = rn_dram

    singles = ctx.enter_context(tc.tile_pool(name="singles", bufs=1))
    g.pool_x = ctx.enter_context(tc.tile_pool(name="pool_x", bufs=2, space="PSUM"))
    g.pool_o = ctx.enter_context(tc.tile_pool(name="pool_o", bufs=2, space="PSUM"))
    g.utpool = ctx.enter_context(tc.tile_pool(name="utpool", bufs=6))
    g.rnpool = ctx.enter_context(tc.tile_pool(name="rnpool", bufs=2))
    g.outp = ctx.enter_context(tc.tile_pool(name="outp", bufs=3))

    nc.gpsimd.load_library(library_config.attn)

    # tri mask first (sync queue): the pre-warm burst depends only on memset
    g.tri_sb = singles.tile([128, 128], BF16, name="tri_sb")
    nc.sync.dma_start(out=g.tri_sb, in_=ins["tri"].ap())
    g.warm_sb = singles.tile([128, 512], F32, name="warm_sb")
    nc.vector.memset(g.warm_sb, 0.5)
    _emit_prewarm(nc, g)

    # --- inputs -----------------------------------------------------------
    # x quarters for tblocks 0,1 land first (the ~3MiB first-mass rides the
    # ~270GB/s effective DMA roofline); the t>=1024 halves follow. Weights
    # rotate over all three DMA-capable queues.
    q_sx = [nc.sync, nc.scalar]
    q3 = [nc.sync, nc.scalar, nc.gpsimd]

    g.x_sb = [singles.tile([128, T], IN_DT, name=f"x{c}") for c in range(8)]
    g.wq_sb = [singles.tile([128, DLOC], IN_DT, name=f"wq{c}") for c in range(8)]
    g.wk_sb = [singles.tile([128, DLOC], IN_DT, name=f"wk{c}") for c in range(8)]
    g.wv_sb = [singles.tile([128, DLOC], IN_DT, name=f"wv{c}") for c in range(8)]
    xT_r = ins["xT"].ap().rearrange("(c p) t -> p c t", p=128)
    wq_src = ins["wq"].ap().rearrange("(c p) m -> p c m", p=128)
    wk_src = ins["wk"].ap().rearrange("(c p) m -> p c m", p=128)
    wv_src = ins["wv"].ap().rearrange("(c p) m -> p c m", p=128)
    for cc in range(8):
        q_sx[cc % 2].dma_start(out=g.x_sb[cc][:, 0:512], in_=xT_r[:, cc, 0:512])
        q3[(cc + 0) % 3].dma_start(out=g.wq_sb[cc], in_=wq_src[:, cc, :])
        q3[(cc + 1) % 3].dma_start(out=g.wk_sb[cc], in_=wk_src[:, cc, :])
        q3[(cc + 2) % 3].dma_start(out=g.wv_sb[cc], in_=wv_src[:, cc, :])

    g.bq_sb = singles.tile([128, 2], F32, name="bq_sb")
    g.bk_sb = singles.tile([128, 2], F32, name="bk_sb")
    g.bvb_sb = singles.tile([128, DLOC], F32, name="bvb_sb")
    g.mneg_sb = singles.tile([128, NKC], F32, name="mneg_sb")
    nc.sync.dma_start(out=g.bq_sb, in_=ins["bq"].ap().rearrange("i p -> p i"))
    nc.scalar.dma_start(out=g.bk_sb, in_=ins["bk"].ap().rearrange("i p -> p i"))
    nc.sync.dma_start(out=g.bvb_sb, in_=ins["bv"].ap().partition_broadcast(128))
    nc.scalar.dma_start(out=g.mneg_sb, in_=ins["mneg"].ap())
    for cc in range(8):
        q_sx[cc % 2].dma_start(
            out=g.x_sb[cc][:, 512:1024], in_=xT_r[:, cc, 512:1024]
        )
    for cc in range(8):
        q_sx[cc % 2].dma_start(out=g.x_sb[cc][:, 1024:T], in_=xT_r[:, cc, 1024:T])

    # --- persistent activations -----------------------------------------
    g.qT_sb = singles.tile([128, 2, T], BF16, tag="qT", name="qT_sb")
    g.kT_sb = singles.tile([128, 2, T], BF16, tag="kT", name="kT_sb")
    g.vaug = [
        singles.tile([128, NKC, VAUG_W], BF16, tag=f"vaug{h}", name=f"vaug{h}")
        for h in range(4)
    ]
    for h in range(4):
        # col 64 = 1.0 -> psum row 64 = softmax denominator; cols 65:96 = 0
        # -> psum rows 65:96 = 0 (initialized, so the denominator transpose
        # reads no garbage)
        nc.vector.memset(g.vaug[h][:, :, D], 1.0)
        nc.vector.memset(g.vaug[h][:, :, D + 1 : VAUG_W], 0.0)
    g.yT_sb = singles.tile([128, 2, T], IN_DT, tag="yT", name="yT_sb")
    g.stg = {i: singles.tile([128, C], F32, name=f"stg{i}") for i in range(8, 16)}
    g.obs = {}

    # --- proj braid units -------------------------------------------------
    def ob_for(i):
        if i not in g.obs:
            g.obs[i] = g.outp.tile([128, C], OUT_DT, tag="ob", name=f"ob{i}")
        return g.obs[i]

    def ob_flush(i, queue):
        ob = g.obs.pop(i)
        queue.dma_start(out=out.ap()[i * 128 : (i + 1) * 128, :], in_=ob)

    def proj_full_half(i, hf):
        """tiles 0..7: both chunks of a 512-col output half -> ob fp16."""
        c0 = hf * 512
        psp = g.pool_x.tile([128, 512], F32, tag="px", name="psp")
        for step, ic in enumerate((1, 0)):
            nc.tensor.matmul(
                psp,
                g.yT_sb[:, ic, i * 128 : (i + 1) * 128],
                g.wp_sb[:, ic, c0 : c0 + 512],
                start=(step == 0),
                stop=(step == 1),
            )
        nc.vector.tensor_copy(ob_for(i)[:, c0 : c0 + 512], psp)
        if hf == 1:
            ob_flush(i, nc.gpsimd)

    def stage_half(i, hf):
        """tiles 8..15: chunk ic=1 partial -> fp32 stage."""
        c0 = hf * 512
        psp = g.pool_x.tile([128, 512], F32, tag="px", name="psp")
        nc.tensor.matmul(
            psp,
            g.yT_sb[:, 1, i * 128 : (i + 1) * 128],
            g.wp_sb[:, 1, c0 : c0 + 512],
            start=True,
            stop=True,
        )
        nc.vector.tensor_copy(g.stg[i][:, c0 : c0 + 512], psp)

    def finish_half(i, hf, eng, queue):
        """tiles 8..15: chunk ic=0 + staged ic=1 -> ob fp16."""
        c0 = hf * 512
        psp = g.pool_x.tile([128, 512], F32, tag="px", name="psp")
        nc.tensor.matmul(
            psp,
            g.yT_sb[:, 0, i * 128 : (i + 1) * 128],
            g.wp_sb[:, 0, c0 : c0 + 512],
            start=True,
            stop=True,
        )
        eng.tensor_tensor(
            out=ob_for(i)[:, c0 : c0 + 512],
            in0=psp,
            in1=g.stg[i][:, c0 : c0 + 512],
            op=AluOpType.add,
        )
        if hf == 1:
            ob_flush(i, queue)

    # --- schedule ---------------------------------------------------------
    _emit_qkv_tblock(nc, g, 0)
    _emit_qkv_tblock(nc, g, 1)
    _emit_attention_single(nc, g, 2, 0)
    _emit_qkv_tblock(nc, g, 2)
    _emit_attention_single(nc, g, 3, 0)
    _emit_qkv_tblock(nc, g, 3)
    # c_proj weights (sync queue is idle from here; needed by pair braids)
    g.wp_sb = singles.tile([128, 2, C], IN_DT, name="wp_sb")
    nc.sync.dma_start(
        out=g.wp_sb, in_=ins["wproj"].ap().rearrange("(i p) n -> p i n", p=128)
    )
    _emit_attention_pair(nc, g, 0, 1, 0)

    mk = lambda f, *a: (lambda: f(*a))
    # pair1: a dense full-half burst at j=3..6 re-warms the HAM clock after
    # the braid-less (0,1|m=0) pair, then stages for tiles 8..15
    units_03 = [mk(proj_full_half, i, hf) for i in range(0, 4) for hf in (0, 1)]
    units_47 = [mk(proj_full_half, i, hf) for i in range(4, 8) for hf in (0, 1)]
    units_stage = [mk(stage_half, i, hf) for i in range(8, 16) for hf in (0, 1)]
    units_fin_a = [
        mk(finish_half, i, hf, nc.vector, nc.gpsimd)
        for i in range(8, 12)
        for hf in (0, 1)
    ]
    _emit_attention_pair(
        nc, g, 2, 3, 1,
        braid=_spread(units_03, 3, 6) | _spread(units_47, 7, 15),
    )
    _emit_attention_pair(
        nc, g, 0, 1, 1,
        braid=_spread(units_stage, 2, 13) | _spread(units_fin_a, 15, 16),
    )
    # tail: finish the last four tiles
    for i in range(12, 16):
        for hf in (0, 1):
            finish_half(i, hf, nc.vector, nc.sync)


@functools.lru_cache(maxsize=1)
def _program():
    nc = bacc.Bacc("TRN2", target_bir_lowering=False, debug=False)
    shapes = {
        "xT": ([C, T], IN_DT),
        "wq": ([C, DLOC], IN_DT),
        "wk": ([C, DLOC], IN_DT),
        "wv": ([C, DLOC], IN_DT),
        "bq": ([2, 128], F32),
        "bk": ([2, 128], F32),
        "bv": ([1, DLOC], F32),
        "wproj": ([DLOC, C], IN_DT),
        "mneg": ([128, NKC], F32),
        "tri": ([128, 128], BF16),
    }
    ins = {
        name: nc.dram_tensor(name, shape, dt_, kind="ExternalInput")
        for name, (shape, dt_) in shapes.items()
    }
    out = nc.dram_tensor("out", [T, C], OUT_DT, kind="ExternalOutput")
    rn_dram = nc.dram_tensor("rn_scratch", [32, 512], F32, kind="Internal")
    with tile.TileContext(nc) as tc, contextlib.ExitStack() as ctx:
        _build(ctx, nc, tc, ins, out, rn_dram)
    nc.compile()
    return nc


def make_in_maps(x, attention_mask, W_attn, b_attn, W_proj, b_proj):
    import ml_dtypes

    in_np = ml_dtypes.bfloat16
    x = np.ascontiguousarray(np.asarray(x, dtype=np.float32))
    attention_mask = np.asarray(attention_mask, dtype=np.float32)
    W_attn = np.asarray(W_attn, dtype=np.float32)
    b_attn = np.asarray(b_attn, dtype=np.float32)
    W_proj = np.asarray(W_proj, dtype=np.float32)

    tri = (np.arange(128)[None, :] >= np.arange(128)[:, None]).astype(np.float32)
    in_maps = []
    for c in range(NCORES):
        b = c // 4
        g = c % 4
        cols = slice(g * DLOC, (g + 1) * DLOC)
        xT = np.ascontiguousarray(x[b].T.astype(in_np))
        mneg = np.ascontiguousarray((attention_mask[b] * NEG).reshape(NKC, 128).T)
        in_maps.append(
            {
                "xT": xT,
                "wq": np.ascontiguousarray(W_attn[:, cols].astype(in_np)),
                "wk": np.ascontiguousarray(W_attn[:, C : 2 * C][:, cols].astype(in_np)),
                "wv": np.ascontiguousarray(
                    W_attn[:, 2 * C : 3 * C][:, cols].astype(in_np)
                ),
                "bq": np.ascontiguousarray(b_attn[cols].reshape(2, 128)),
                "bk": np.ascontiguousarray(b_attn[C : 2 * C][cols].reshape(2, 128)),
                "bv": np.ascontiguousarray(b_attn[2 * C : 3 * C][cols].reshape(1, DLOC)),
                "wproj": np.ascontiguousarray(
                    W_proj[g * DLOC : (g + 1) * DLOC, :].astype(in_np)
                ),
                "mneg": mneg,
                "tri": tri.astype(in_np),
            }
        )
    return in_maps


def kernel(x, attention_mask, W_attn, b_attn, W_proj, b_proj, _res_hook=None):
    in_maps = make_in_maps(x, attention_mask, W_attn, b_attn, W_proj, b_proj)
    nc = _program()
    res = bass_utils.run_bass_kernel_spmd(nc, in_maps, core_ids=list(range(NCORES)))
    if _res_hook is not None:
        _res_hook(res)
    b_proj = np.asarray(b_proj, dtype=np.float32)
    y = np.zeros((B, T, C), dtype=np.float32)
    for c in range(NCORES):
        y[c // 4] += np.asarray(res.results[c]["out"], dtype=np.float32)
    y += b_proj[None, None, :]
    return y
